# revision 1
# baseline (speedup 1.0000x reference)
"""Trainium2 Bass kernel for nn_Attention: GPT-2 style attention block.

Per-core work (data-parallel over batch, 1 of 8 batch elements per core):
  qkv = x @ wa + ba ; split q,k,v into 12 heads of 64
  S_h = q_h @ k_h^T            (no 1/sqrt(D) scaling)
  S masked multiplicatively with tril (masked entries ~= 0, still in softmax!)
  P = softmax(S) ; a_h = P @ v_h
  merged[t, d*12+h] = a_h[t, d] ; y = merged @ wp + bp

Key implementation ideas:
  - Big matmuls in float32r (full fp32 bits, fast PE path for N>=256).
  - Scores computed twice: once in [i,j] orientation (row stats only), once in
    [j,i] orientation (for the AV matmul with P^T as moving operand) with
    -(rowmax + lnZ) folded in via K=1 accumulate matmuls, so the exp output is
    already normalized (no per-row division anywhere).
  - Masked-position weights e^{-L_i}: in-diagonal-block wedge fixed with
    copy_predicated from a broadcast tile; fully-masked column blocks feed the
    AV matmul directly from that broadcast tile (scores never materialized).
  - Masked-count contribution to Z enters as one extra ln(count) column in the
    stats reduce + exp-accumulate pass.
  - wp rows permuted at load time to absorb the interleaved (D,H) merge; AV
    head pairs col-packed with tile_position so PSUM is directly the merged
    layout chunk.
  - P^T, v, merged, wp2 in bf16 (values O(1); final rel-err budget allows it).
"""

import math
import sys

sys.path.insert(0, "/opt/trn_rl_repo")

import numpy as np

import concourse.bass as bass
from concourse import bacc
import concourse.mybir as mybir
import concourse.tile as tile
from concourse import bass_utils
from concourse.masks import make_identity

F32 = mybir.dt.float32
F32R = mybir.dt.float32r
BF16 = mybir.dt.bfloat16
AF = mybir.ActivationFunctionType
ALU = mybir.AluOpType

T = 1024
C = 768
H = 12
D = 64
NT = T // 128        # 8 T-chunks
NCC = C // 128       # 6 C-chunks
# PT storage: per head, block b holds score cols [128*b, 1024), width 1024-128b
PT_W = [T - 128 * b for b in range(NT)]
PT_OFF = [sum(PT_W[:b]) for b in range(NT)]
PT_TOT = sum(PT_W)   # 4608


def r32(ap):
    return ap.bitcast(F32R)


def _patch_act_tables():
    from concourse import bacc as _bacc_mod
    import concourse.hw_specs as _hw
    if getattr(_bacc_mod, "_act_tables_patched", False):
        return
    orig = _bacc_mod.get_activation_tables

    def one_set(arch):
        t = orig(arch)
        keep = "natural_log_exp_and_others"
        if keep in t:
            t = {k: (v if k == keep else set()) for k, v in t.items()}
        return t

    _bacc_mod.get_activation_tables = one_set
    _bacc_mod._act_tables_patched = True


def build_nc():
    _patch_act_tables()
    nc = bacc.Bacc("TRN2", target_bir_lowering=False, debug=False, num_devices=8)

    x = nc.dram_tensor("x", [T, C], F32, kind="ExternalInput").ap()
    wa = nc.dram_tensor("wa", [C, 3 * C], F32, kind="ExternalInput").ap()
    ba = nc.dram_tensor("ba", [3 * C], F32, kind="ExternalInput").ap()
    wp = nc.dram_tensor("wp", [C, C], F32, kind="ExternalInput").ap()
    bp = nc.dram_tensor("bp", [C], F32, kind="ExternalInput").ap()
    y = nc.dram_tensor("y", [T, C], F32, kind="ExternalOutput").ap()

    with tile.TileContext(nc) as tc:
        build_attention(tc, x, wa, ba, wp, bp, y)
    nc.compile()
    return nc


def build_attention(tc, x, wa, ba, wp, bp, y):
    nc = tc.nc

    with (
        tc.tile_pool(name="consts", bufs=1) as consts,
        tc.tile_pool(name="persist", bufs=1) as persist,
        tc.tile_pool(name="rows", bufs=1) as rows,
    ):
        # ---------------- constants ----------------
        masks = consts.tile([128, 384], F32, tag="masks")
        ident = masks[:, 0:128]
        tril = masks[:, 128:256]
        make_identity(nc, ident)
        # tril[p, f] = 1 where f <= p (keep), else 0   ([i-part, j-free])
        nc.gpsimd.memset(tril, 1.0)
        nc.gpsimd.affine_select(
            out=tril, in_=tril, compare_op=ALU.is_ge, fill=0.0,
            base=0, pattern=[[-1, 128]], channel_multiplier=1,
        )
        # wedge[p, f] = 1 where p > f  ([j-part, i-free]: masked region j > i)
        wedge = consts.tile([128, 128], mybir.dt.int8, tag="wedge")
        nc.gpsimd.memset(wedge, 1)
        nc.gpsimd.affine_select(
            out=wedge, in_=wedge, compare_op=ALU.is_gt, fill=0,
            base=0, pattern=[[-1, 128]], channel_multiplier=1,
        )
        onesf = consts.tile([2, 128], F32, tag="onesf")
        nc.vector.memset(onesf, 1.0)
        ones2 = consts.tile([2, 128], F32R, tag="ones2")
        nc.scalar.copy(ones2, onesf)
        onesrow = ones2[0:1, :]
        onesb = consts.tile([1, 128], BF16, tag="onesb")
        nc.vector.memset(onesb, 1.0)
        # ln of half the masked-count beyond computed region, per row-chunk r
        # (two columns per r so the fp32r matmul has N=2)
        lncf = consts.tile([1, 2 * NT], F32, tag="lncf")
        nc.vector.memset(lncf, 0.0)
        for r in range(NT - 1):
            nc.vector.memset(
                lncf[:, 2 * r : 2 * r + 2], math.log((T - 128 * (r + 1)) / 2)
            )
        lnc = consts.tile([1, 2 * NT], F32R, tag="lnc")
        nc.scalar.copy(lnc, lncf)
        # bav in [0:768], bp in [768:1536]
        bavbp = consts.tile([1, 2 * C], F32R, tag="bavbp")
        nc.scalar.dma_start(
            out=bavbp[:, 0:C], in_=ba[2 * C : 3 * C].rearrange("(a c) -> a c", a=1).bitcast(F32R)
        )
        nc.scalar.dma_start(
            out=bavbp[:, C : 2 * C], in_=bp.rearrange("(a c) -> a c", a=1).bitcast(F32R)
        )
        bpb = consts.tile([1, C], BF16, tag="bpb")
        nc.scalar.copy(bpb, bavbp[:, C : 2 * C].bitcast(F32))
        # per-partition bias for q/k projection copies: col m = ba[128m:128(m+1)]
        ba_qk = consts.tile([128, 2 * NCC], F32, tag="ba_qk")
        nc.scalar.dma_start(
            out=ba_qk, in_=ba[0 : 2 * C].rearrange("(m p) -> p m", p=128)
        )

        # ---------------- persistent activations ----------------
        qkT = persist.tile([128, 2 * NCC, T], F32R, tag="qkT")  # chunks: 0-5 q, 6-11 k
        v_sb = persist.tile([128, NT, C], BF16, tag="v_sb")    # [t-part, tchunk, feat]
        negLst = persist.tile([128, 8 * H], F32, tag="negLst")  # col 8h+r

        # ---------------- phases: load/transpose; proj+stats interleaved; attn --
        stall = persist.tile([128, 6 * 48], F32, tag="stall")
        wp2 = persist.tile([128, NCC, C], F32R, tag="wp2")
        v_suf = persist.tile([128, 7, C], BF16, tag="v_suf")
        with (
            tc.tile_pool(name="stpsum", bufs=2, space="PSUM") as stpsum,
            tc.tile_pool(name="stsb", bufs=2) as stsb,
        ):
            def emit_stats(pair):
                stpair = stall[:, 48 * pair : 48 * pair + 48]
                for half in range(2):
                    h = 2 * pair + half
                    qm, qp = h // 2, (h % 2) * 64
                    negm = stpair[:, 8 * half : 8 * half + 8]
                    zst = stpair[:, 16 + 8 * half : 24 + 8 * half]
                    lnz = stpair[:, 32 + 8 * half : 40 + 8 * half]
                    for r in range(NT):
                        jcols = 128 * (r + 1)
                        sps = stpsum.tile([128, 1024], F32, tag="ps_s")
                        pieces = [(0, min(jcols, 512))]
                        if jcols > 512:
                            pieces.append((512, jcols - 512))
                        for (p0, pw) in pieces:
                            nc.tensor.matmul(
                                sps[:, p0 : p0 + pw],
                                qkT[qp : qp + 64, qm, 128 * r : 128 * r + 128],
                                qkT[qp : qp + 64, 6 + qm, p0 : p0 + pw],
                                start=True,
                                stop=True,
                            )
                        ncols = jcols
                        if r < NT - 1:
                            # 2 extra cols = ln(count/2) -> exp adds count*e^{-m}
                            nc.tensor.matmul(
                                sps[:, jcols : jcols + 2],
                                onesrow,
                                lnc[:, 2 * r : 2 * r + 2],
                                start=True,
                                stop=True,
                            )
                            ncols = jcols + 2
                        # causal mask on the diagonal 128x128 block
                        nc.vector.tensor_mul(
                            sps[:, 128 * r : 128 * r + 128],
                            sps[:, 128 * r : 128 * r + 128],
                            tril,
                        )
                        nc.vector.reduce_max(
                            negm[:, r : r + 1], sps[:, 0:ncols],
                            axis=mybir.AxisListType.X, negate=True,
                        )
                        scratch = stsb.tile([128, 1026], BF16, tag="scratch")
                        nc.scalar.activation(
                            scratch[:, 0:ncols], sps[:, 0:ncols], AF.Exp,
                            bias=negm[:, r : r + 1],
                            accum_out=zst[:, r : r + 1],
                        )
                    nc.scalar.activation(lnz, zst, AF.Ln)
                    nc.vector.tensor_sub(
                        stpair[:, 8 * half : 8 * half + 8], negm, lnz
                    )
                return stpair

            with (
                tc.tile_pool(name="xload", bufs=1) as xload,
                tc.tile_pool(name="xstream", bufs=4) as xstream,
                tc.tile_pool(name="ph1psum", bufs=2, space="PSUM") as ph1psum,
            ):
                wa_sb = xload.tile([128, NCC, 3 * C], F32R, tag="wa_sb")
                xT = xload.tile([128, NCC, T], F32R, tag="xT")

                def emit_xt(trange):
                    for t in trange:
                        xc = xstream.tile([128, C], F32, tag="xchunk")
                        nc.sync.dma_start(
                            out=xc, in_=x[128 * t : 128 * t + 128, :]
                        )
                        for g in range(2):
                            ps = ph1psum.tile([128, 384], F32, tag="ps_misc")
                            for q in range(3):
                                cc = 3 * g + q
                                nc.tensor.transpose(
                                    ps[:, 128 * q : 128 * q + 128],
                                    xc[:, 128 * cc : 128 * cc + 128], ident,
                                )
                            nc.vector.tensor_copy(
                                xT[:, 3 * g : 3 * g + 3, 128 * t : 128 * t + 128],
                                ps,
                            )

                def emit_projqk(p, ns=(0, 1)):
                    for m in (p, 6 + p):
                        for n in ns:
                            ps = ph1psum.tile([128, 512], F32, tag="ps_proj")
                            for cc in range(NCC):
                                nc.tensor.matmul(
                                    ps,
                                    wa_sb[:, cc, 128 * m : 128 * m + 128],
                                    xT[:, cc, 512 * n : 512 * n + 512],
                                    start=(cc == 0),
                                    stop=(cc == NCC - 1),
                                )
                            nc.scalar.activation(
                                qkT[:, m, 512 * n : 512 * n + 512], ps,
                                AF.Identity, bias=ba_qk[:, m : m + 1],
                            )

                def emit_vproj():
                    for t in range(NT):
                        for n in range(2):
                            ps = ph1psum.tile([128, 384], F32, tag="ps_misc")
                            for cc in range(NCC):
                                nc.tensor.matmul(
                                    ps,
                                    xT[:, cc, 128 * t : 128 * t + 128],
                                    wa_sb[:, cc,
                                          2 * C + 384 * n : 2 * C + 384 * n + 384],
                                    start=(cc == 0),
                                    stop=False,
                                )
                            nc.tensor.matmul(
                                ps,
                                onesrow,
                                bavbp[:, 384 * n : 384 * n + 384],
                                start=False,
                                stop=True,
                            )
                            nc.vector.tensor_copy(
                                v_sb[:, t, 384 * n : 384 * n + 384], ps
                            )

                emit_xt(range(2))
                for cc in range(2):
                    nc.scalar.dma_start(
                        out=wa_sb[:, cc, :],
                        in_=wa[128 * cc : 128 * cc + 128, :].bitcast(F32R),
                    )
                emit_xt(range(2, 4))
                for cc in range(2, NCC):
                    nc.scalar.dma_start(
                        out=wa_sb[:, cc, :],
                        in_=wa[128 * cc : 128 * cc + 128, :].bitcast(F32R),
                    )
                for p in range(6):
                    emit_projqk(p, ns=(0,))
                emit_xt(range(4, NT))
                emit_projqk(0, ns=(1,))
                emit_stats(0)
                emit_projqk(1, ns=(1,))
                emit_stats(1)
                emit_vproj()
                # v_suf[m] = sum of v blocks b > m (for masked-region AV)
                nc.vector.tensor_copy(v_suf[:, 6, :], v_sb[:, 7, :])
                for m in range(5, -1, -1):
                    nc.vector.tensor_add(
                        v_suf[:, m, :], v_suf[:, m + 1, :], v_sb[:, m + 1, :]
                    )
                for p in range(2, 6):
                    emit_projqk(p, ns=(1,))

            # -------- attn phase: per-pair rows -> P^T -> AV -------------------
            with (
                tc.tile_pool(name="rowp", bufs=2) as rowp,
                tc.tile_pool(name="rowh", bufs=2) as rowh,
                tc.tile_pool(name="ptpool", bufs=2) as ptpool,
                tc.tile_pool(name="bexpool", bufs=1) as bexpool,
                tc.tile_pool(name="avpsum", bufs=2, space="PSUM") as avpsum,
                tc.tile_pool(name="stps2", bufs=2, space="PSUM") as stps2,
                tc.tile_pool(name="ph23", bufs=1) as ph23,
                tc.tile_pool(name="ysb", bufs=2) as ysb,
            ):
                mergedT = ph23.tile([128, NCC, T], F32R, tag="mergedT")
                # wp2 load (row-permuted: merged col c2=h*64+d <-> wp row d*12+h)
                wp_r = wp.rearrange("(d h) c -> d h c", h=H)  # [64, 12, 768]
                for k in range(NCC):
                    wst = ysb.tile([128, C], F32, tag="y_stage")
                    for par in range(2):
                        nc.sync.dma_start(
                            out=wst[64 * par : 64 * par + 64, :],
                            in_=wp_r[:, 2 * k + par, :],
                        )
                    nc.scalar.copy(wp2[:, k, :], wst)

                def emit_rows(pair, stpair):
                    rowf = rowp.tile([16, 384], F32, tag="rowf")
                    negLp = rowf[:, 0:128]
                    negLphi = rowf[:, 128:256].bitcast(F32R)
                    negLplo = rowf[:, 256:384].bitcast(F32R)
                    expLp = rowp.tile([16, 128], BF16, tag="expLp")
                    erowp = rowp.tile([1, 2 * T], BF16, tag="erowp")
                    pst = stps2.tile([128, 512], F32, tag="ps_st")
                    nc.tensor.transpose(pst[0:48, 0:128], stpair, ident)
                    nc.scalar.copy(negLp, pst[0:16, 0:128])
                    nc.scalar.copy(negLphi, negLp)
                    nc.vector.tensor_sub(negLplo, negLp, negLphi.bitcast(F32))
                    nc.scalar.activation(expLp, negLp, AF.Exp)
                    nc.sync.dma_start(
                        out=erowp.rearrange("a (p f) -> a p f", p=16), in_=expLp
                    )
                    return negLphi, negLplo, erowp

                def emit_attn(pair, rowsinfo):
                    negLphi, negLplo, erowp = rowsinfo
                    pts = []
                    bexps = []
                    for half in range(2):
                        h = 2 * pair + half
                        qm, qp = h // 2, (h % 2) * 64
                        nlr = rowh.tile([2, T], F32R, tag="nlr")
                        nc.sync.dma_start(
                            out=nlr[0:1, :].rearrange("a (p f) -> a p f", p=8),
                            in_=negLphi[8 * half : 8 * half + 8, :],
                        )
                        nc.sync.dma_start(
                            out=nlr[1:2, :].rearrange("a (p f) -> a p f", p=8),
                            in_=negLplo[8 * half : 8 * half + 8, :],
                        )
                        bexp = bexpool.tile([128, T], BF16, tag=f"bexp{half}")
                        nc.gpsimd.partition_broadcast(
                            bexp, erowp[:, T * half : T * half + T], channels=128
                        )
                        pt = ptpool.tile([128, PT_TOT], BF16, tag=f"pt{half}")
                        for b in range(NT):
                            if b < 4:
                                pieces = [(128 * b, 512 - 128 * b), (512, 512)]
                            else:
                                pieces = [(128 * b, T - 128 * b)]
                            for (g0, gw) in pieces:
                                ps = stps2.tile([128, 512], F32, tag="ps_st")
                                nc.tensor.matmul(
                                    ps[:, 0:gw],
                                    qkT[qp : qp + 64, 6 + qm, 128 * b : 128 * b + 128],
                                    qkT[qp : qp + 64, qm, g0 : g0 + gw],
                                    start=True,
                                    stop=False,
                                )
                                # fold in -(max + lnZ) along the free (i) axis
                                nc.tensor.matmul(
                                    ps[:, 0:gw],
                                    ones2,
                                    nlr[:, g0 : g0 + gw],
                                    start=False,
                                    stop=True,
                                )
                                lo = PT_OFF[b] + g0 - 128 * b
                                nc.scalar.activation(
                                    pt[:, lo : lo + gw], ps[:, 0:gw], AF.Exp
                                )
                            # wedge of diag block -> e^{-L_i}
                            nc.vector.copy_predicated(
                                pt[:, PT_OFF[b] : PT_OFF[b] + 128],
                                wedge,
                                bexp[:, 128 * b : 128 * b + 128],
                            )
                        pts.append(pt)
                        bexps.append(bexp)

                    # AV: out chunk = [headA d (part 0-63) | headB d (part 64-127)]
                    # valid region from PT; masked region: for col range
                    # [128m, 128m+128) all blocks b>m contribute e^{-L_i} * v_b,
                    # i.e. one v_suf[m]^T @ Bexp matmul per range.
                    for c in range(2):
                        ps = avpsum.tile([128, 512], F32, tag="ps_av")
                        for half in range(2):
                            h = 2 * pair + half
                            mms = []
                            for b in range(NT):
                                lo_blk = 128 * b
                                c0, c1 = 512 * c, 512 * c + 512
                                if lo_blk >= c1:
                                    continue
                                g0 = max(lo_blk, c0)
                                lo = PT_OFF[b] + g0 - lo_blk
                                mms.append(
                                    (v_sb[:, b, 64 * h : 64 * h + 64],
                                     pts[half][:, lo : lo + (c1 - g0)], g0 - c0)
                                )
                            for m in range(4 * c, min(4 * c + 4, 7)):
                                mms.append(
                                    (v_suf[:, m, 64 * h : 64 * h + 64],
                                     bexps[half][:, 128 * m : 128 * m + 128],
                                     128 * m - 512 * c)
                                )
                            for idx, (lhsT, rhs, off) in enumerate(mms):
                                nw = rhs.shape[-1]
                                nc.tensor.matmul(
                                    ps[64 * half : 64 * half + 64, off : off + nw],
                                    lhsT, rhs,
                                    start=(idx == 0),
                                    stop=(idx == len(mms) - 1),
                                    tile_position=(0, 64 * half),
                                    skip_group_check=True,
                                )
                        nc.vector.tensor_copy(
                            mergedT[:, pair, 512 * c : 512 * c + 512], ps
                        )

                rinfo = {0: emit_rows(0, stall[:, 0:48])}
                for p in range(6):
                    if p + 2 < 6:
                        emit_stats(p + 2)
                    if p + 1 < 6:
                        rinfo[p + 1] = emit_rows(
                            p + 1, stall[:, 48 * (p + 1) : 48 * (p + 1) + 48]
                        )
                    emit_attn(p, rinfo.pop(p))

                # ---------------- phase 3: c_proj --------------------------------
                for t in range(NT):
                    yt = ysb.tile([128, C], F32, tag="y_stage")
                    for (n0, nw) in ((0, 512), (512, 256)):
                        ps = avpsum.tile([128, 512], F32, tag="ps_av")
                        for k in range(NCC):
                            nc.tensor.matmul(
                                ps[:, 0:nw],
                                mergedT[:, k, 128 * t : 128 * t + 128],
                                wp2[:, k, n0 : n0 + nw],
                                start=(k == 0),
                                stop=False,
                            )
                        nc.tensor.matmul(
                            ps[:, 0:nw],
                            onesrow,
                            bavbp[:, C + n0 : C + n0 + nw],
                            start=False,
                            stop=True,
                        )
                        nc.vector.tensor_copy(yt[:, n0 : n0 + nw], ps[:, 0:nw])
                    nc.sync.dma_start(out=y[128 * t : 128 * t + 128, :], in_=yt)


_NC_CACHE = None


def get_nc():
    global _NC_CACHE
    if _NC_CACHE is None:
        _NC_CACHE = build_nc()
    return _NC_CACHE


def kernel(x, wa, ba, wp, bp, **kw):
    x = np.asarray(x, dtype=np.float32)
    in_maps = [
        {
            "x": np.ascontiguousarray(x[b]),
            "wa": np.asarray(wa, dtype=np.float32),
            "ba": np.asarray(ba, dtype=np.float32),
            "wp": np.asarray(wp, dtype=np.float32),
            "bp": np.asarray(bp, dtype=np.float32),
        }
        for b in range(8)
    ]
    res = bass_utils.run_bass_kernel_spmd(get_nc(), in_maps, core_ids=list(range(8)))
    return np.stack([r["y"] for r in res.results], axis=0)


if __name__ == "__main__":
    nc = build_nc()
    print("build OK, instructions:", sum(1 for _ in nc.m.functions[0].body) if hasattr(nc.m.functions[0], "body") else "n/a")



# revision 13
# speedup vs baseline: 1.0425x; 1.0425x over previous
"""Trainium2 Bass kernel for nn_Attention: GPT-2 style attention block.

Data-parallel over batch: core b computes batch element b (8 cores, B=8).

Per-core algorithm (T=1024, C=768, H=12, D=64):
  qkv = x @ wa + ba ; per head: S = q k^T (no 1/sqrt(D));
  S masked multiplicatively with tril (masked entries ~0 STILL in softmax);
  P = softmax(S); a = P v; merged (D,H)-interleaved; y = merged @ wp + bp.

Implementation (v2 — late-Z normalization, single-exp):
  - Host pre-transposes/pre-permutes all weights (xt, wa slices, wp row-perm)
    so the device does zero layout work.
  - Stats pass computes ONLY the per-row max m_i (no Z/lnZ): one fp32r score
    pass in [i,j] orientation, fused causal-mask+max via DVE
    tensor_mask_reduce with accum_in=0.0 (the masked entries' exp(~0)
    candidates give max >= 0, matching the reference's multiplicative mask).
  - P^T pass: scores in [j,i] orientation with the -m_i fold FUSED into the
    matmul via 65-row augmented q/k tiles (row 64: ones on the k side,
    -m_i on the q side) -> exp gives unnormalized U^T = e^{s-m} directly.
  - Z comes free through the AV matmul: v is stored in 65-channel head
    groups whose 65th channel is 1.0, so AV psum row 64 = sum_j U^T = Z
    (masked regions enter via the v_suf suffix-sum trick and the
    copy_predicated diagonal wedge fill with e^{-m}).
  - Final normalization: one DVE divide per (head, 512-chunk) writing
    mergedT (odd heads stage + DMA partition-shift).
  - c_proj with host-row-permuted wp in bf16 (merged also bf16).
"""

import math
import sys

sys.path.insert(0, "/opt/trn_rl_repo")

import numpy as np

import concourse.bass as bass
from concourse import bacc
import concourse.mybir as mybir
import concourse.tile as tile
from concourse import bass_utils
from concourse.masks import make_identity

F32 = mybir.dt.float32
F32R = mybir.dt.float32r
BF16 = mybir.dt.bfloat16
U16 = mybir.dt.uint16
AF = mybir.ActivationFunctionType
ALU = mybir.AluOpType

T = 1024
C = 768
H = 12
D = 64
NCC = C // 128       # 6
NT = T // 128        # 8
VW = H * (D + 1)     # 780: v stored as 12 head-groups of (64 d + 1 ones)
HV = VW // 2         # 390
EARLY = 5            # heads whose stats run during phase 1

# pt layout: paired blocks [b0 | b1 b7 | b2 b6 | b3 b5 | b4] so each psum
# group is a full [128, 1024] (or 512) tile -> one exp per group.
PT_GROUPS = [(0,), (1, 7), (2, 6), (3, 5), (4,)]
PT_W = [T - 128 * b for b in range(NT)]
PT_OFF = {}
_off = 0
for _g in PT_GROUPS:
    for _b in _g:
        PT_OFF[_b] = _off
        _off += PT_W[_b]
PT_TOT = _off        # 4608

# PT matmul pieces per block, in i coordinates (start, width); <=512 per
# piece and no piece crossing a psum bank boundary within its group.
PT_PIECES = {
    0: [(0, 512), (512, 512)],
    1: [(128, 512), (640, 384)],
    7: [(896, 128)],
    2: [(256, 512), (768, 256)],
    6: [(768, 256)],
    3: [(384, 512), (896, 128)],
    5: [(640, 384)],
    4: [(512, 512)],
}

# stats pieces: (r, j0, w, me_col)
# me cols: 0:p+1 1:p+129 2:p+257 3:p+385 4:p+193 5:p+321 6:320 7:384 8:448 9:512
STATS_PIECES = [
    (0, 0, 128, 0),
    (1, 0, 256, 1),
    (2, 0, 384, 2),
    (3, 0, 512, 3),
    (4, 0, 320, 6), (4, 320, 320, 4),
    (5, 0, 384, 7), (5, 384, 384, 2),
    (6, 0, 448, 8), (6, 448, 448, 5),
    (7, 0, 512, 9), (7, 512, 512, 3),
]


def r32(ap):
    return ap.bitcast(F32R)


def _patch_act_tables():
    from concourse import bacc as _bacc_mod
    if getattr(_bacc_mod, "_act_tables_patched", False):
        return
    orig = _bacc_mod.get_activation_tables

    def one_set(arch):
        t = orig(arch)
        keep = "natural_log_exp_and_others"
        if keep in t:
            t = {k: (v if k == keep else set()) for k, v in t.items()}
        return t

    _bacc_mod.get_activation_tables = one_set
    _bacc_mod._act_tables_patched = True


def build_nc():
    _patch_act_tables()
    nc = bacc.Bacc("TRN2", target_bir_lowering=False, debug=False, num_devices=8)

    xt = nc.dram_tensor("xt", [C, T], F32, kind="ExternalInput").ap()
    waqk = nc.dram_tensor("waqk", [C, 2 * C], F32, kind="ExternalInput").ap()
    wav = nc.dram_tensor("wav", [C, VW], F32, kind="ExternalInput").ap()
    baqk = nc.dram_tensor("baqk", [128, H], F32, kind="ExternalInput").ap()
    bav = nc.dram_tensor("bav", [1, VW], F32, kind="ExternalInput").ap()
    wp2h = nc.dram_tensor("wp2h", [128, NCC * C], U16, kind="ExternalInput").ap()
    bph = nc.dram_tensor("bph", [1, C], F32, kind="ExternalInput").ap()
    trilh = nc.dram_tensor("trilh", [128, 128], F32, kind="ExternalInput").ap()
    onesh = nc.dram_tensor("onesh", [1, H * T], F32, kind="ExternalInput").ap()
    meh = nc.dram_tensor("meh", [128, 10], F32, kind="ExternalInput").ap()
    y = nc.dram_tensor("y", [T, C], F32, kind="ExternalOutput").ap()

    with tile.TileContext(nc) as tc:
        build_attention(tc, xt, waqk, wav, baqk, bav, wp2h, bph, trilh, meh, onesh, y)
    nc.compile()
    return nc


def build_attention(tc, xt, waqk, wav, baqk, bav, wp2h, bph, trilh, meh, onesh, y):
    nc = tc.nc

    with (
        tc.tile_pool(name="consts", bufs=1) as consts,
        tc.tile_pool(name="persist", bufs=1) as persist,
        tc.tile_pool(name="rowsp", bufs=2) as rowsp,
        tc.tile_pool(name="stagep", bufs=2) as stagep,
    ):
        # ---------------- constants ----------------
        ident = consts.tile([128, 128], F32, tag="ident")
        make_identity(nc, ident)
        me = consts.tile([128, 10], F32, tag="me")
        nc.sync.dma_start(out=me, in_=meh)
        baqk_sb = consts.tile([128, H], F32, tag="baqk")
        nc.sync.dma_start(out=baqk_sb, in_=baqk)
        bav_row = consts.tile([1, VW], F32R, tag="bavrow")
        nc.sync.dma_start(out=bav_row, in_=bav.bitcast(F32R))
        bp_row = consts.tile([1, C], F32R, tag="bprow")
        nc.sync.dma_start(out=bp_row, in_=bph.bitcast(F32R))
        onesf = consts.tile([1, 128], F32, tag="onesf")
        nc.vector.memset(onesf, 1.0)
        onesr = consts.tile([1, 128], F32R, tag="onesr")
        nc.scalar.copy(onesr, onesf)
        # wedge[p(j), f(i)] = 1 where j > i (masked region of diagonal block)
        wedge = consts.tile([128, 128], mybir.dt.int8, tag="wedge")
        nc.gpsimd.memset(wedge, 1)
        nc.gpsimd.affine_select(
            out=wedge, in_=wedge, compare_op=ALU.is_gt, fill=0,
            base=0, pattern=[[-1, 128]], channel_multiplier=1,
        )

        # ---------------- persistent activations ----------------
        # augmented q/k: per head [65, 1024]; row 64 = -m_i (q) / ones (k)
        qa = persist.tile([65, H, T], F32R, tag="qa")
        ka = persist.tile([65, H, T], F32R, tag="ka")
        nc.sync.dma_start(
            out=ka[64:65, :, :],
            in_=onesh.bitcast(F32R).rearrange("a (h t) -> a h t", h=H),
        )
        v2 = persist.tile([128, NT, VW], BF16, tag="v2")
        vsuf = persist.tile([128, NT - 1, VW], BF16, tag="vsuf")
        negm = persist.tile([128, H * NT], F32, tag="negm")   # col 8h+r

        def emit_stats_mm(h, psum_pool):
            """Score matmuls [i,j] orientation for head h -> psum pieces."""
            tiles = []
            for (r, j0, w, mc) in STATS_PIECES:
                sps = psum_pool.tile([128, 512], F32, tag="st_ps")
                nc.tensor.matmul(
                    sps[:, 0:w],
                    qa[0:64, h, 128 * r: 128 * r + 128],
                    ka[0:64, h, j0: j0 + w],
                    start=True, stop=True,
                )
                tiles.append((r, j0, w, mc, sps))
            return tiles

        def emit_stats_red(h, tiles):
            """Fused causal-mask + row-max via DVE tensor_mask_reduce."""
            i = 0
            while i < len(tiles):
                r, j0, w, mc, sps = tiles[i]
                two = i + 1 < len(tiles) and tiles[i + 1][0] == r
                if not two:
                    nc.vector.tensor_mask_reduce(
                        out=sps[:, 0:w], in_=sps[:, 0:w],
                        mask_start=0.0, mask_end=me[:, mc: mc + 1],
                        scale=1.0, accum_in=0.0, op=ALU.max,
                        negate_accum=True,
                        accum_out=negm[:, 8 * h + r: 8 * h + r + 1],
                    )
                    i += 1
                else:
                    tmp = rowsp.tile([128, 1], F32, tag="tmpmax")
                    nc.vector.tensor_mask_reduce(
                        out=sps[:, 0:w], in_=sps[:, 0:w],
                        mask_start=0.0, mask_end=me[:, mc: mc + 1],
                        scale=1.0, accum_in=0.0, op=ALU.max,
                        negate_accum=False, accum_out=tmp,
                    )
                    r2, j02, w2, mc2, sps2 = tiles[i + 1]
                    nc.vector.tensor_mask_reduce(
                        out=sps2[:, 0:w2], in_=sps2[:, 0:w2],
                        mask_start=0.0, mask_end=me[:, mc2: mc2 + 1],
                        scale=1.0, accum_in=tmp, op=ALU.max,
                        negate_accum=True,
                        accum_out=negm[:, 8 * h + r: 8 * h + r + 1],
                    )
                    i += 2

        def emit_stats(h, psum_pool):
            emit_stats_red(h, emit_stats_mm(h, psum_pool))

        # ---------------- phase 1: projections + early stats -----------------
        with tc.tile_pool(name="xtp", bufs=1) as xtp:
            xT = xtp.tile([128, NCC, T], F32R, tag="xT")
            for cc in range(NCC):
                for n in range(2):
                    nc.sync.dma_start(
                        out=xT[:, cc, 512 * n: 512 * n + 512],
                        in_=r32(xt[128 * cc: 128 * cc + 128, 512 * n: 512 * n + 512]),
                    )

            with (
                tc.tile_pool(name="wqk", bufs=1) as wqk,
                tc.tile_pool(name="ph1psum", bufs=3, space="PSUM") as ph1psum,
            ):
                wa_sb = wqk.tile([128, NCC, 2 * C], F32R, tag="wa_sb")
                for cc in range(NCC):
                    nc.sync.dma_start(
                        out=wa_sb[:, cc, :],
                        in_=r32(waqk[128 * cc: 128 * cc + 128, :]),
                    )

                def emit_projqk(m, n):
                    # m 0..5 q chunks, 6..11 k chunks; heads 2*(m%6)(+1)
                    dest = qa if m < 6 else ka
                    ps = ph1psum.tile([128, 512], F32, tag="pqk")
                    for cc in range(NCC):
                        nc.tensor.matmul(
                            ps,
                            wa_sb[:, cc, 128 * m: 128 * m + 128],
                            xT[:, cc, 512 * n: 512 * n + 512],
                            start=(cc == 0), stop=(cc == NCC - 1),
                        )
                    hA = 2 * (m % 6)
                    # head A: rows 0:64 straight down (Act, bias add)
                    nc.scalar.activation(
                        dest[0:64, hA, 512 * n: 512 * n + 512],
                        ps[0:64, :], AF.Identity,
                        bias=baqk_sb[0:64, m: m + 1],
                    )
                    # head B: rows 64:128 -> stage (Act, bias) -> DMA shift
                    st = stagep.tile([128, 512], F32R, tag="qkstage")
                    nc.scalar.activation(
                        st[64:128, :], ps[64:128, :], AF.Identity,
                        bias=baqk_sb[64:128, m: m + 1],
                    )
                    nc.gpsimd.dma_start(
                        out=dest[0:64, hA + 1, 512 * n: 512 * n + 512],
                        in_=st[64:128, :],
                    )

                for m in range(12):
                    for n in range(2):
                        emit_projqk(m, n)

            with (
                tc.tile_pool(name="wvp", bufs=1) as wvp,
                tc.tile_pool(name="ph1bpsum", bufs=3, space="PSUM") as ph1bpsum,
            ):
                wav_sb = wvp.tile([128, NCC, VW], F32R, tag="wav_sb")
                for cc in range(NCC):
                    nc.sync.dma_start(
                        out=wav_sb[:, cc, :],
                        in_=r32(wav[128 * cc: 128 * cc + 128, :]),
                    )
                for tb in range(NT):
                    for nn in range(2):
                        ps = ph1bpsum.tile([128, HV], F32, tag="pv")
                        for cc in range(NCC):
                            nc.tensor.matmul(
                                ps,
                                xT[:, cc, 128 * tb: 128 * tb + 128],
                                wav_sb[:, cc, HV * nn: HV * nn + HV],
                                start=(cc == 0), stop=False,
                            )
                        # bias + ones-channel fold: += 1 * bav[f]
                        nc.tensor.matmul(
                            ps, onesr,
                            bav_row[:, HV * nn: HV * nn + HV],
                            start=False, stop=True,
                        )
                        nc.scalar.copy(v2[:, tb, HV * nn: HV * nn + HV], ps)
                    if tb < EARLY:
                        emit_stats(tb, ph1bpsum)
                # v_suf[m] = sum of v2 blocks b > m (gpsimd: SBUF bf16)
                nc.gpsimd.tensor_copy(vsuf[:, 6, :], v2[:, 7, :])
                for m in range(5, -1, -1):
                    nc.gpsimd.tensor_add(
                        vsuf[:, m, :], vsuf[:, m + 1, :], v2[:, m + 1, :]
                    )

        # -------- phase 2: per-head stats -> rows -> P^T -> AV -> norm --------
        with (
            tc.tile_pool(name="ph2", bufs=1) as ph2,
            tc.tile_pool(name="bexpp", bufs=2) as bexpp,
            tc.tile_pool(name="zbp", bufs=2) as zbp,
            tc.tile_pool(name="stpsum", bufs=2, space="PSUM") as stpsum,
            tc.tile_pool(name="ptpsum", bufs=2, space="PSUM") as ptpsum,
            tc.tile_pool(name="avpsum", bufs=2, space="PSUM") as avpsum,
            tc.tile_pool(name="ysbp", bufs=2) as ysbp,
        ):
            mergedT = ph2.tile([128, NCC, T], BF16, tag="mergedT")
            wp2 = ph2.tile([128, NCC, C], BF16, tag="wp2")
            pt_all = ph2.tile([128, 2, PT_TOT], BF16, tag="pt")
            # wp2 load (host pre-permuted, bf16 bits)
            nc.sync.dma_start(out=wp2, in_=wp2h.bitcast(BF16))
            def emit_rows(h):
                """negm[:, 8h:8h+8] -> qa aug row (-m) + bexp broadcast."""
                ps = stpsum.tile([128, 512], F32, tag="st_ps")
                nc.tensor.transpose(
                    ps[0:8, 0:128], negm[:, 8 * h: 8 * h + 8], ident
                )
                expn = rowsp.tile([8, 128], BF16, tag="expn")
                nc.scalar.activation(expn, ps[0:8, 0:128], AF.Exp)
                negmT = rowsp.tile([8, 128], F32R, tag="negmT")
                nc.vector.tensor_copy(negmT, ps[0:8, 0:128])
                nc.gpsimd.dma_start(
                    out=qa[64:65, h, :].rearrange("a (p f) -> a p f", p=8),
                    in_=negmT,
                )
                erow = rowsp.tile([1, T], BF16, tag="erow")
                nc.gpsimd.dma_start(
                    out=erow.rearrange("a (p f) -> a p f", p=8), in_=expn
                )
                bexp = bexpp.tile([128, T], BF16, tag="bexp")
                nc.gpsimd.partition_broadcast(bexp, erow, channels=128)
                return bexp

            def emit_pt(h, bexp):
                """U^T = exp(qk^T - m) via augmented matmul; wedge <- e^{-m}."""
                pt = pt_all[:, h % 2, :]
                for grp in PT_GROUPS:
                    goff = PT_OFF[grp[0]]
                    gw = sum(PT_W[b] for b in grp)
                    pps = ptpsum.tile([128, 1024], F32, tag="pt_ps")
                    for b in grp:
                        for (i0, w) in PT_PIECES[b]:
                            lo = PT_OFF[b] + i0 - 128 * b - goff
                            nc.tensor.matmul(
                                pps[:, lo: lo + w],
                                ka[0:65, h, 128 * b: 128 * b + 128],
                                qa[0:65, h, i0: i0 + w],
                                start=True, stop=True,
                            )
                    nc.scalar.activation(
                        pt[:, goff: goff + gw], pps[:, 0:gw], AF.Exp,
                    )
                for b in range(NT):
                    nc.vector.copy_predicated(
                        pt[:, PT_OFF[b]: PT_OFF[b] + 128],
                        wedge,
                        bexp[:, 128 * b: 128 * b + 128],
                    )
                return pt

            def emit_av(h, pt, bexp):
                """AV with ones-channel Z row; [65, 512] psums."""
                avts = []
                for c in range(2):
                    c0, c1 = 512 * c, 512 * c + 512
                    avt = avpsum.tile([128, 512], F32, tag="av_ps")
                    mms = []
                    for b in range(NT):
                        if 128 * b >= c1:
                            continue
                        g0 = max(128 * b, c0)
                        lo = PT_OFF[b] + g0 - 128 * b
                        mms.append((v2[:, b, 65 * h: 65 * h + 65],
                                    pt[:, lo: lo + (c1 - g0)], g0 - c0))
                    for m in range(4 * c, min(4 * c + 4, 7)):
                        mms.append((vsuf[:, m, 65 * h: 65 * h + 65],
                                    bexp[:, 128 * m: 128 * m + 128],
                                    128 * m - c0))
                    for idx, (lhsT, rhs, o0) in enumerate(mms):
                        nw = rhs.shape[-1]
                        nc.tensor.matmul(
                            avt[0:65, o0: o0 + nw], lhsT, rhs,
                            start=(idx == 0), stop=(idx == len(mms) - 1),
                            skip_group_check=True,
                        )
                    avts.append(avt)
                return avts

            def emit_norm(h, avts):
                """mergedT[rows(h), h//2, :] = avt[0:64]/Z ; Z = avt row 64."""
                k = h // 2
                for c in range(2):
                    avt = avts[c]
                    cs = slice(512 * c, 512 * c + 512)
                    zrow = rowsp.tile([1, 512], F32, tag="zrow")
                    nc.scalar.copy(zrow, avt[64:65, 0:512])
                    zri = rowsp.tile([1, 512], F32, tag="zri")
                    nc.vector.reciprocal(zri, zrow)
                    zb = zbp.tile([64, 512], F32, tag="zb")
                    nc.gpsimd.partition_broadcast(zb, zri, channels=64)
                    if h % 2 == 0:
                        nc.vector.tensor_tensor(
                            out=mergedT[0:64, k, cs], in0=avt[0:64, :], in1=zb,
                            op=ALU.mult,
                        )
                    else:
                        dst = stagep.tile([64, 512], BF16, tag="divstage")
                        nc.vector.tensor_tensor(
                            out=dst, in0=avt[0:64, :], in1=zb, op=ALU.mult,
                        )
                        nc.gpsimd.dma_start(out=mergedT[64:128, k, cs], in_=dst)

            bexps = {}
            for i in range(13):
                if EARLY <= i + 4 < 12:
                    st_tiles = emit_stats_mm(i + 4, stpsum)
                if i < 12:
                    bexps[i] = emit_rows(i)
                if i - 1 >= 0:
                    h = i - 1
                    pt = emit_pt(h, bexps[h])
                    avts = emit_av(h, pt, bexps.pop(h))
                    emit_norm(h, avts)
                if EARLY <= i + 4 < 12:
                    emit_stats_red(i + 4, st_tiles)

            # ---------------- phase 3: c_proj --------------------------------
            for tb in range(NT):
                yt = ysbp.tile([128, C], F32, tag="y_stage")
                for (n0, nw) in ((0, 512), (512, 256)):
                    ps = avpsum.tile([128, 512], F32, tag="av_ps")
                    for k in range(NCC):
                        nc.tensor.matmul(
                            ps[:, 0:nw],
                            mergedT[:, k, 128 * tb: 128 * tb + 128],
                            wp2[:, k, n0: n0 + nw],
                            start=(k == 0), stop=False,
                        )
                    # bias fold: += 1 * bp[n]
                    nc.tensor.matmul(
                        ps[:, 0:nw], onesr,
                        bp_row[:, n0: n0 + nw],
                        start=False, stop=True,
                    )
                    nc.scalar.copy(yt[:, n0: n0 + nw], ps[:, 0:nw])
                nc.sync.dma_start(out=y[128 * tb: 128 * tb + 128, :], in_=yt)


# ---------------- host side ----------------

def _bf16_bits(a):
    u = np.ascontiguousarray(a, dtype=np.float32).view(np.uint32)
    r = (u >> 16) & 1
    return ((u + 0x7FFF + r) >> 16).astype(np.uint16)


def _prep_shared(wa, ba, wp, bp):
    wa = np.asarray(wa, dtype=np.float32)
    ba = np.asarray(ba, dtype=np.float32)
    wp = np.asarray(wp, dtype=np.float32)
    bp = np.asarray(bp, dtype=np.float32)
    waqk = np.ascontiguousarray(wa[:, : 2 * C])
    wav = np.zeros((C, VW), dtype=np.float32)
    wav.reshape(C, H, D + 1)[:, :, :D] = wa[:, 2 * C:].reshape(C, H, D)
    bav = np.zeros((1, VW), dtype=np.float32)
    bav.reshape(H, D + 1)[:, :D] = ba[2 * C:].reshape(H, D)
    bav.reshape(H, D + 1)[:, D] = 1.0
    baqk = np.ascontiguousarray(ba[: 2 * C].reshape(12, 128).T)
    wpr = wp.reshape(D, H, C)
    wp2h = np.empty((128, NCC, C), dtype=np.float32)
    for k in range(NCC):
        wp2h[0:64, k] = wpr[:, 2 * k]
        wp2h[64:128, k] = wpr[:, 2 * k + 1]
    wp2h = _bf16_bits(wp2h.reshape(128, NCC * C))
    bph = bp.reshape(1, C)
    p = np.arange(128, dtype=np.float32).reshape(128, 1)
    me = np.concatenate(
        [p + 1, p + 129, p + 257, p + 385, p + 193, p + 321,
         np.full((128, 1), 320.0, np.float32), np.full((128, 1), 384.0, np.float32),
         np.full((128, 1), 448.0, np.float32), np.full((128, 1), 512.0, np.float32)],
        axis=1).astype(np.float32)
    tr = np.tril(np.ones((128, 128), dtype=np.float32))
    return {
        "waqk": waqk, "wav": wav, "bav": bav, "baqk": np.ascontiguousarray(baqk),
        "wp2h": wp2h, "bph": np.ascontiguousarray(bph), "meh": me, "trilh": tr,
        "onesh": np.ones((1, H * T), dtype=np.float32),
    }


_NC_CACHE = None


def get_nc():
    global _NC_CACHE
    if _NC_CACHE is None:
        _NC_CACHE = build_nc()
    return _NC_CACHE


def kernel(x, wa, ba, wp, bp, **kw):
    x = np.asarray(x, dtype=np.float32)
    shared = _prep_shared(wa, ba, wp, bp)
    in_maps = [
        dict(shared, xt=np.ascontiguousarray(x[b].T)) for b in range(8)
    ]
    res = bass_utils.run_bass_kernel_spmd(get_nc(), in_maps, core_ids=list(range(8)))
    return np.stack([r["y"] for r in res.results], axis=0)


if __name__ == "__main__":
    nc = build_nc()
    print("build OK")


# revision 26
# speedup vs baseline: 1.1642x; 1.1168x over previous
"""Trainium2 Bass kernel for nn_Attention: GPT-2 style attention block.

Data-parallel over batch: core b computes batch element b (8 cores, B=8).

Per-core algorithm (T=1024, C=768, H=12, D=64):
  qkv = x @ wa + ba ; per head: S = q k^T (no 1/sqrt(D));
  S masked multiplicatively with tril (masked entries ~0 STILL in softmax);
  P = softmax(S); a = P v; merged (D,H)-interleaved; y = merged @ wp + bp.

Implementation (v2 — late-Z normalization, single-exp):
  - Host pre-transposes/pre-permutes all weights (xt, wa slices, wp row-perm)
    so the device does zero layout work.
  - Stats pass computes ONLY the per-row max m_i (no Z/lnZ): one fp32r score
    pass in [i,j] orientation, fused causal-mask+max via DVE
    tensor_mask_reduce with accum_in=0.0 (the masked entries' exp(~0)
    candidates give max >= 0, matching the reference's multiplicative mask).
  - P^T pass: scores in [j,i] orientation with the -m_i fold FUSED into the
    matmul via 65-row augmented q/k tiles (row 64: ones on the k side,
    -m_i on the q side) -> exp gives unnormalized U^T = e^{s-m} directly.
  - Z comes free through the AV matmul: v is stored in 65-channel head
    groups whose 65th channel is 1.0, so AV psum row 64 = sum_j U^T = Z
    (masked regions enter via the v_suf suffix-sum trick and the
    copy_predicated diagonal wedge fill with e^{-m}).
  - Final normalization: one DVE divide per (head, 512-chunk) writing
    mergedT (odd heads stage + DMA partition-shift).
  - c_proj with host-row-permuted wp in bf16 (merged also bf16).
"""

import math
import sys

sys.path.insert(0, "/opt/trn_rl_repo")

import numpy as np

import concourse.bass as bass
from concourse import bacc
import concourse.mybir as mybir
import concourse.tile as tile
from concourse import bass_utils
from concourse.masks import make_identity

F32 = mybir.dt.float32
F32R = mybir.dt.float32r
BF16 = mybir.dt.bfloat16
U16 = mybir.dt.uint16
AF = mybir.ActivationFunctionType
ALU = mybir.AluOpType

T = 1024
C = 768
H = 12
D = 64
NCC = C // 128       # 6
NT = T // 128        # 8
VW = H * (D + 1)     # 780: v stored as 12 head-groups of (64 d + 1 ones)
HV = VW // 2         # 390
EARLY = 5            # heads whose stats run during phase 1

# pt layout: paired blocks [b0 | b1 b7 | b2 b6 | b3 b5 | b4] so each psum
# group is a full [128, 1024] (or 512) tile -> one exp per group.
PT_GROUPS = [(0,), (7, 3, 6), (1,), (2,), (4, 5)]
PT_W = [T - 128 * b for b in range(NT)]
PT_OFF = {}
_off = 0
for _g in PT_GROUPS:
    for _b in _g:
        PT_OFF[_b] = _off
        _off += PT_W[_b]
PT_TOT = _off        # 4608

# PT matmul pieces per block, in i coordinates (start, width); <=512 per
# piece and no piece crossing a psum bank boundary within its group.
PT_PIECES = {
    0: [(0, 512), (512, 512)],
    1: [(128, 512), (640, 384)],
    7: [(896, 128)],
    2: [(256, 512), (768, 256)],
    6: [(768, 256)],
    3: [(384, 384), (768, 256)],
    5: [(640, 384)],
    4: [(512, 512)],
}

# stats pieces: (r, j0, w, diag_local_start or None)
STATS_PIECES = [
    (0, 0, 128, 0),
    (1, 0, 256, 128),
    (2, 0, 384, 256),
    (3, 0, 512, 384),
    (4, 0, 320, None), (4, 320, 320, 192),
    (5, 0, 384, None), (5, 384, 384, 256),
    (6, 0, 448, None), (6, 448, 448, 320),
    (7, 0, 512, None), (7, 512, 512, 384),
]


def r32(ap):
    return ap.bitcast(F32R)


def _patch_act_tables():
    from concourse import bacc as _bacc_mod
    if getattr(_bacc_mod, "_act_tables_patched", False):
        return
    orig = _bacc_mod.get_activation_tables

    def one_set(arch):
        t = orig(arch)
        keep = "natural_log_exp_and_others"
        if keep in t:
            t = {k: (v if k == keep else set()) for k, v in t.items()}
        return t

    _bacc_mod.get_activation_tables = one_set
    _bacc_mod._act_tables_patched = True


def build_nc():
    _patch_act_tables()
    nc = bacc.Bacc("TRN2", target_bir_lowering=False, debug=False, num_devices=8)

    xt = nc.dram_tensor("xt", [C, T], F32, kind="ExternalInput").ap()
    waqk = nc.dram_tensor("waqk", [C, 2 * C], F32, kind="ExternalInput").ap()
    wav = nc.dram_tensor("wav", [C, VW], F32, kind="ExternalInput").ap()
    baqk = nc.dram_tensor("baqk", [128, H], F32, kind="ExternalInput").ap()
    bav = nc.dram_tensor("bav", [1, VW], F32, kind="ExternalInput").ap()
    wp2h = nc.dram_tensor("wp2h", [128, NCC * C], U16, kind="ExternalInput").ap()
    bph = nc.dram_tensor("bph", [1, C], F32, kind="ExternalInput").ap()
    trilh = nc.dram_tensor("trilh", [128, 128], F32, kind="ExternalInput").ap()
    onesh = nc.dram_tensor("onesh", [1, H * T], F32, kind="ExternalInput").ap()
    meh = nc.dram_tensor("meh", [128, 10], F32, kind="ExternalInput").ap()
    y = nc.dram_tensor("y", [T, C], F32, kind="ExternalOutput").ap()

    with tile.TileContext(nc) as tc:
        build_attention(tc, xt, waqk, wav, baqk, bav, wp2h, bph, trilh, meh, onesh, y)
    nc.compile()
    return nc


def build_attention(tc, xt, waqk, wav, baqk, bav, wp2h, bph, trilh, meh, onesh, y):
    nc = tc.nc

    with (
        tc.tile_pool(name="consts", bufs=1) as consts,
        tc.tile_pool(name="persist", bufs=1) as persist,
        tc.tile_pool(name="rowsp", bufs=2) as rowsp,
        tc.tile_pool(name="stagep", bufs=2) as stagep,
    ):
        # ---------------- constants ----------------
        ident = consts.tile([128, 128], F32, tag="ident")
        make_identity(nc, ident)
        me = consts.tile([128, 10], F32, tag="me")
        nc.scalar.dma_start(out=me, in_=meh)
        baqk_sb = consts.tile([128, H], F32, tag="baqk")
        nc.scalar.dma_start(out=baqk_sb, in_=baqk)
        bav_row = consts.tile([1, VW], F32R, tag="bavrow")
        nc.sync.dma_start(out=bav_row, in_=bav.bitcast(F32R))

        nc.sync.dma_start(out=bp_row, in_=bph.bitcast(F32R))
        onesf = consts.tile([1, 128], F32, tag="onesf")
        nc.vector.memset(onesf, 1.0)
        onesr = consts.tile([1, 128], F32R, tag="onesr")
        nc.scalar.copy(onesr, onesf)
        # wedge[p(j), f(i)] = 1 where j > i (masked region of diagonal block)
        wedge = consts.tile([128, 128], mybir.dt.int8, tag="wedge")
        nc.gpsimd.memset(wedge, 1)
        nc.gpsimd.affine_select(
            out=wedge, in_=wedge, compare_op=ALU.is_gt, fill=0,
            base=0, pattern=[[-1, 128]], channel_multiplier=1,
        )

        # ---------------- persistent activations ----------------
        # augmented q/k: per head [65, 1024]; row 64 = -m_i (q) / ones (k)
        qa = persist.tile([65, H, T], F32R, tag="qa")
        ka = persist.tile([65, H, T], F32R, tag="ka")
        v2 = persist.tile([128, NT, VW], BF16, tag="v2")
        vsuf = persist.tile([128, NT - 1, VW], BF16, tag="vsuf")
        negm = persist.tile([128, H * NT], F32, tag="negm")   # col 8h+r

        def emit_stats_mm(h, psum_pool):
            """Score matmuls [i,j] orientation for head h -> psum pieces."""
            tiles = []
            for (r, j0, w, mc) in STATS_PIECES:
                sps = psum_pool.tile([128, 512], F32, tag="st_ps")
                nc.tensor.matmul(
                    sps[:, 0:w],
                    qa[0:64, h, 128 * r: 128 * r + 128],
                    ka[0:64, h, j0: j0 + w],
                    start=True, stop=True,
                )
                tiles.append((r, j0, w, mc, sps))
            return tiles

        def emit_stats_red(h, tiles):
            """Fused causal-mask + row-max via DVE tensor_mask_reduce."""
            i = 0
            while i < len(tiles):
                r, j0, w, mc, sps = tiles[i]
                two = i + 1 < len(tiles) and tiles[i + 1][0] == r
                if not two:
                    nc.vector.tensor_mask_reduce(
                        out=sps[:, 0:w], in_=sps[:, 0:w],
                        mask_start=0.0, mask_end=me[:, mc: mc + 1],
                        scale=1.0, accum_in=0.0, op=ALU.max,
                        negate_accum=True,
                        accum_out=negm[:, 8 * h + r: 8 * h + r + 1],
                    )
                    i += 1
                else:
                    tmp = rowsp.tile([128, 1], F32, tag="tmpmax")
                    nc.vector.tensor_mask_reduce(
                        out=sps[:, 0:w], in_=sps[:, 0:w],
                        mask_start=0.0, mask_end=me[:, mc: mc + 1],
                        scale=1.0, accum_in=0.0, op=ALU.max,
                        negate_accum=False, accum_out=tmp,
                    )
                    r2, j02, w2, mc2, sps2 = tiles[i + 1]
                    nc.vector.tensor_mask_reduce(
                        out=sps2[:, 0:w2], in_=sps2[:, 0:w2],
                        mask_start=0.0, mask_end=me[:, mc2: mc2 + 1],
                        scale=1.0, accum_in=tmp, op=ALU.max,
                        negate_accum=True,
                        accum_out=negm[:, 8 * h + r: 8 * h + r + 1],
                    )
                    i += 2

        ptpsum_ref = [None]

        def stats_piece_emitters(h, psum_pool):
            "One callback per stats piece: matmul + fused mask/max reduce."
            ems = []
            state = {}

            def mk(idx):
                def go():
                    r, j0, w, mc = STATS_PIECES[idx]
                    shape = [128, 512] if psum_pool is not ptpsum_ref[0] else [128, 1024]
                    sps = psum_pool.tile(shape, F32, tag=(
                        "st_ps" if psum_pool is not ptpsum_ref[0] else "pt_ps"))
                    nc.tensor.matmul(
                        sps[:, 0:w],
                        qa[0:64, h, 128 * r: 128 * r + 128],
                        ka[0:64, h, j0: j0 + w],
                        start=True, stop=True,
                    )
                    two = idx + 1 < len(STATS_PIECES) and STATS_PIECES[idx + 1][0] == r
                    first = idx == 0 or STATS_PIECES[idx - 1][0] != r
                    acc_in = 0.0 if first else state.pop("tmp")
                    if two:
                        tmp = rowsp.tile([128, 1], F32, tag="tmpmax")
                        state["tmp"] = tmp
                        nc.vector.tensor_mask_reduce(
                            out=sps[:, 0:w], in_=sps[:, 0:w],
                            mask_start=0.0, mask_end=me[:, mc: mc + 1],
                            scale=1.0, accum_in=acc_in, op=ALU.max,
                            negate_accum=False, accum_out=tmp,
                        )
                    else:
                        nc.vector.tensor_mask_reduce(
                            out=sps[:, 0:w], in_=sps[:, 0:w],
                            mask_start=0.0, mask_end=me[:, mc: mc + 1],
                            scale=1.0, accum_in=acc_in, op=ALU.max,
                            negate_accum=True,
                            accum_out=negm[:, 8 * h + r: 8 * h + r + 1],
                        )
                return go

            for idx in range(len(STATS_PIECES)):
                ems.append(mk(idx))
            return ems

        def emit_stats(h, psum_pool):
            for em in stats_piece_emitters(h, psum_pool):
                em()

        # ---------------- phase 1: projections + early stats -----------------
        with tc.tile_pool(name="xtp", bufs=1) as xtp:
            xT = xtp.tile([128, NCC, T], F32R, tag="xT")
            xt_loads = [
                (nc.sync if cc % 2 == 0 else nc.scalar, cc) for cc in range(NCC)
            ]
            for eng, cc in xt_loads:
                eng.dma_start(
                    out=xT[:, cc, :],
                    in_=r32(xt[128 * cc: 128 * cc + 128, :]),
                )

            with (
                tc.tile_pool(name="wqk", bufs=1) as wqk,
                tc.tile_pool(name="wvp", bufs=1) as wvp,
                tc.tile_pool(name="ph1psum", bufs=4, space="PSUM") as ph1psum,
            ):
                wa_sb = wqk.tile([128, NCC, 2 * C], F32R, tag="wa_sb")
                for cc in range(NCC):
                    eng = nc.scalar if cc % 2 == 0 else nc.sync
                    eng.dma_start(
                        out=wa_sb[:, cc, :],
                        in_=r32(waqk[128 * cc: 128 * cc + 128, :]),
                    )
                wav_sb = wvp.tile([128, NCC, VW], F32R, tag="wav_sb")
                for cc in range(NCC):
                    eng = nc.scalar if cc % 2 == 0 else nc.sync
                    eng.dma_start(
                        out=wav_sb[:, cc, :],
                        in_=r32(wav[128 * cc: 128 * cc + 128, :]),
                    )

                def emit_projqk(m, n):
                    # m 0..5 q chunks, 6..11 k chunks; heads 2*(m%6)(+1)
                    dest = qa if m < 6 else ka
                    ps = ph1pj.tile([128, 512], F32, tag="pqk")
                    for cc in range(NCC):
                        nc.tensor.matmul(
                            ps,
                            wa_sb[:, cc, 128 * m: 128 * m + 128],
                            xT[:, cc, 512 * n: 512 * n + 512],
                            start=(cc == 0), stop=(cc == NCC - 1),
                        )
                    hA = 2 * (m % 6)
                    # head A: rows 0:64 straight down (Act, bias add)
                    nc.scalar.activation(
                        dest[0:64, hA, 512 * n: 512 * n + 512],
                        ps[0:64, :], AF.Identity,
                        bias=baqk_sb[0:64, m: m + 1],
                    )
                    # head B: rows 64:128 -> stage (Act, bias) -> DMA shift
                    st = stagep.tile([128, 512], F32R, tag="qkstage")
                    nc.scalar.activation(
                        st[64:128, :], ps[64:128, :], AF.Identity,
                        bias=baqk_sb[64:128, m: m + 1],
                    )
                    nc.scalar.dma_start(
                        out=dest[0:64, hA + 1, 512 * n: 512 * n + 512],
                        in_=st[64:128, :],
                    )

                from collections import deque
                pieceq = deque()
                for mp in range(6):
                    for m in (mp, 6 + mp):
                        for n in range(2):
                            emit_projqk(m, n)
                            for _ in range(4):
                                if pieceq:
                                    pieceq.popleft()()
                    if mp < 3:
                        pieceq.extend(stats_piece_emitters(2 * mp, ph1psum))
                        pieceq.extend(stats_piece_emitters(2 * mp + 1, ph1psum))
                while pieceq:
                    pieceq.popleft()()

                from collections import deque as _dq
                pieceq2 = _dq()
                for hh in (6, 7, 8, 9):
                    pieceq2.extend(stats_piece_emitters(hh, ph1psum))
                for tb in range(NT):
                    for nn in range(2):
                        ps = ph1pv.tile([128, HV], F32, tag="pv")
                        for cc in range(NCC):
                            nc.tensor.matmul(
                                ps,
                                xT[:, cc, 128 * tb: 128 * tb + 128],
                                wav_sb[:, cc, HV * nn: HV * nn + HV],
                                start=(cc == 0), stop=False,
                            )
                        # bias + ones-channel fold: += 1 * bav[f]
                        nc.tensor.matmul(
                            ps, onesr,
                            bav_row[:, HV * nn: HV * nn + HV],
                            start=False, stop=True,
                        )
                        nc.scalar.copy(v2[:, tb, HV * nn: HV * nn + HV], ps)

                # v_suf[m] = sum of v2 blocks b > m (gpsimd: SBUF bf16)
                nc.gpsimd.tensor_copy(vsuf[:, 6, :], v2[:, 7, :])
                for m in range(5, -1, -1):
                    nc.gpsimd.tensor_add(
                        vsuf[:, m, :], vsuf[:, m + 1, :], v2[:, m + 1, :]
                    )

        # -------- phase 2: per-head stats -> rows -> P^T -> AV -> norm --------
        with (
            tc.tile_pool(name="ph2", bufs=1) as ph2,
            tc.tile_pool(name="bexpp", bufs=3) as bexpp,
            tc.tile_pool(name="zbp", bufs=2) as zbp,
            tc.tile_pool(name="ptpsum", bufs=2, space="PSUM") as ptpsum,
            tc.tile_pool(name="avpsum", bufs=4, space="PSUM") as avpsum,
            tc.tile_pool(name="ysbp", bufs=2) as ysbp,
        ):
            ptpsum_ref[0] = ptpsum
            mergedT = ph2.tile([128, NCC, T], BF16, tag="mergedT")
            wp2 = ph2.tile([128, NCC, C], BF16, tag="wp2")
            pt_all = ph2.tile([128, 3, PT_TOT], BF16, tag="pt")
            # wp2 load (host pre-permuted, bf16 bits)
            nc.scalar.dma_start(out=wp2, in_=wp2h.bitcast(BF16))
            def emit_rows(h):
                """negm[:, 8h:8h+8] -> qa aug row (-m) + bexp broadcast."""
                nc.scalar.dma_start(
                    out=ka[64:65, h, :],
                    in_=onesh[:, T * 0: T].bitcast(F32R),
                )
                ps = avpsum.tile([128, 512], F32, tag="av_ps")
                nc.tensor.transpose(
                    ps[0:8, 0:128], negm[:, 8 * h: 8 * h + 8], ident
                )
                expn = rowsp.tile([8, 128], BF16, tag="expn")
                nc.scalar.activation(expn, ps[0:8, 0:128], AF.Exp)
                negmT = rowsp.tile([8, 128], F32R, tag="negmT")
                nc.vector.tensor_copy(negmT, ps[0:8, 0:128])
                nc.sync.dma_start(
                    out=qa[64:65, h, :].rearrange("a (p f) -> a p f", p=8),
                    in_=negmT,
                )
                erow = rowsp.tile([1, T], BF16, tag="erow")
                nc.sync.dma_start(
                    out=erow.rearrange("a (p f) -> a p f", p=8), in_=expn
                )
                bexp = bexpp.tile([128, T], BF16, tag="bexp")
                nc.gpsimd.partition_broadcast(bexp, erow, channels=128)
                return bexp

            def emit_pt_group(h, pt, grp, bexp):
                goff = PT_OFF[grp[0]]
                gw = sum(PT_W[b] for b in grp)
                pps = ptpsum.tile([128, 1024], F32, tag="pt_ps")
                for b in grp:
                    for (i0, w) in PT_PIECES[b]:
                        lo = PT_OFF[b] + i0 - 128 * b - goff
                        nc.tensor.matmul(
                            pps[:, lo: lo + w],
                            ka[0:65, h, 128 * b: 128 * b + 128],
                            qa[0:65, h, i0: i0 + w],
                            start=True, stop=True,
                        )
                nc.scalar.activation(
                    pt[:, goff: goff + gw], pps[:, 0:gw], AF.Exp,
                )
                for b in grp:
                    nc.vector.copy_predicated(
                        pt[:, PT_OFF[b]: PT_OFF[b] + 128],
                        wedge,
                        bexp[:, 128 * b: 128 * b + 128],
                    )

            def emit_av_c(h, pt, bexp, c):
                """AV with ones-channel Z row; [65, 512] psum for chunk c."""
                if True:
                    c0, c1 = 512 * c, 512 * c + 512
                    avt = avpsum.tile([128, 512], F32, tag="av_ps")
                    mms = []
                    for b in range(NT):
                        if 128 * b >= c1:
                            continue
                        g0 = max(128 * b, c0)
                        lo = PT_OFF[b] + g0 - 128 * b
                        mms.append((v2[:, b, 65 * h: 65 * h + 65],
                                    pt[:, lo: lo + (c1 - g0)], g0 - c0))
                    for m in range(4 * c, min(4 * c + 4, 7)):
                        mms.append((vsuf[:, m, 65 * h: 65 * h + 65],
                                    bexp[:, 128 * m: 128 * m + 128],
                                    128 * m - c0))
                    for idx, (lhsT, rhs, o0) in enumerate(mms):
                        nw = rhs.shape[-1]
                        nc.tensor.matmul(
                            avt[0:65, o0: o0 + nw], lhsT, rhs,
                            start=(idx == 0), stop=(idx == len(mms) - 1),
                            skip_group_check=True,
                        )
                    return avt

            def emit_norm(h, avts):
                """mergedT[rows(h), h//2, :] = avt[0:64]/Z ; Z = avt row 64."""
                k = h // 2
                for c in range(2):
                    avt = avts[c]
                    cs = slice(512 * c, 512 * c + 512)
                    zrow = rowsp.tile([1, 512], F32, tag="zrow")
                    nc.scalar.copy(zrow, avt[64:65, 0:512])
                    zri = rowsp.tile([1, 512], F32, tag="zri")
                    nc.vector.reciprocal(zri, zrow)
                    zb = zbp.tile([64, 512], F32, tag="zb")
                    nc.gpsimd.partition_broadcast(zb, zri, channels=64)
                    if h % 2 == 0:
                        nc.vector.tensor_tensor(
                            out=mergedT[0:64, k, cs], in0=avt[0:64, :], in1=zb,
                            op=ALU.mult,
                        )
                    else:
                        dst = stagep.tile([64, 512], BF16, tag="divstage")
                        nc.vector.tensor_tensor(
                            out=dst, in0=avt[0:64, :], in1=zb, op=ALU.mult,
                        )
                        nc.scalar.dma_start(out=mergedT[64:128, k, cs], in_=dst)

            bexps = {}
            from collections import deque as _dq3
            pieceq3 = _dq3()
            pieceq3.extend(stats_piece_emitters(10, ptpsum))
            pieceq3.extend(stats_piece_emitters(11, ptpsum))
            for i in range(13):
                for _ in range(4):
                    if pieceq3:
                        pieceq3.popleft()()
                if i < 12:
                    bexps[i] = emit_rows(i)
                if i - 1 >= 0:
                    h = i - 1
                    bexp = bexps.pop(h)
                    pt = pt_all[:, h % 3, :]
                    for gi in range(4):
                        emit_pt_group(h, pt, PT_GROUPS[gi], bexp)
                    avt0 = emit_av_c(h, pt, bexp, 0)
                    emit_pt_group(h, pt, PT_GROUPS[4], bexp)
                    avt1 = emit_av_c(h, pt, bexp, 1)
                    emit_norm(h, [avt0, avt1])

            # ---------------- phase 3: c_proj --------------------------------
            for tb in range(NT):
                yt = ysbp.tile([128, C], F32, tag="y_stage")
                for (n0, nw) in ((0, 512), (512, 256)):
                    ps = avpsum.tile([128, 512], F32, tag="av_ps")
                    for k in range(NCC):
                        nc.tensor.matmul(
                            ps[:, 0:nw],
                            mergedT[:, k, 128 * tb: 128 * tb + 128],
                            wp2[:, k, n0: n0 + nw],
                            start=(k == 0), stop=False,
                        )
                    # bias fold: += 1 * bp[n]
                    nc.tensor.matmul(
                        ps[:, 0:nw], onesr,
                        bp_row[:, n0: n0 + nw],
                        start=False, stop=True,
                    )
                    nc.scalar.copy(yt[:, n0: n0 + nw], ps[:, 0:nw])
                nc.sync.dma_start(out=y[128 * tb: 128 * tb + 128, :], in_=yt)


# ---------------- host side ----------------

def _bf16_bits(a):
    u = np.ascontiguousarray(a, dtype=np.float32).view(np.uint32)
    r = (u >> 16) & 1
    return ((u + 0x7FFF + r) >> 16).astype(np.uint16)


def _prep_shared(wa, ba, wp, bp):
    wa = np.asarray(wa, dtype=np.float32)
    ba = np.asarray(ba, dtype=np.float32)
    wp = np.asarray(wp, dtype=np.float32)
    bp = np.asarray(bp, dtype=np.float32)
    waqk = np.ascontiguousarray(wa[:, : 2 * C])
    wav = np.zeros((C, VW), dtype=np.float32)
    wav.reshape(C, H, D + 1)[:, :, :D] = wa[:, 2 * C:].reshape(C, H, D)
    bav = np.zeros((1, VW), dtype=np.float32)
    bav.reshape(H, D + 1)[:, :D] = ba[2 * C:].reshape(H, D)
    bav.reshape(H, D + 1)[:, D] = 1.0
    baqk = np.ascontiguousarray(ba[: 2 * C].reshape(12, 128).T)
    wpr = wp.reshape(D, H, C)
    wp2h = np.empty((128, NCC, C), dtype=np.float32)
    for k in range(NCC):
        wp2h[0:64, k] = wpr[:, 2 * k]
        wp2h[64:128, k] = wpr[:, 2 * k + 1]
    wp2h = _bf16_bits(wp2h.reshape(128, NCC * C))
    bph = bp.reshape(1, C)
    p = np.arange(128, dtype=np.float32).reshape(128, 1)
    me = np.concatenate(
        [p + 1, p + 129, p + 257, p + 385, p + 193, p + 321,
         np.full((128, 1), 320.0, np.float32), np.full((128, 1), 384.0, np.float32),
         np.full((128, 1), 448.0, np.float32), np.full((128, 1), 512.0, np.float32)],
        axis=1).astype(np.float32)
    tr = np.tril(np.ones((128, 128), dtype=np.float32))
    return {
        "waqk": waqk, "wav": wav, "bav": bav, "baqk": np.ascontiguousarray(baqk),
        "wp2h": wp2h, "bph": np.ascontiguousarray(bph), "meh": me, "trilh": tr,
        "onesh": np.ones((1, H * T), dtype=np.float32),
    }


_NC_CACHE = None


def get_nc():
    global _NC_CACHE
    if _NC_CACHE is None:
        _NC_CACHE = build_nc()
    return _NC_CACHE


def kernel(x, wa, ba, wp, bp, **kw):
    x = np.asarray(x, dtype=np.float32)
    shared = _prep_shared(wa, ba, wp, bp)
    in_maps = [
        dict(shared, xt=np.ascontiguousarray(x[b].T)) for b in range(8)
    ]
    res = bass_utils.run_bass_kernel_spmd(get_nc(), in_maps, core_ids=list(range(8)))
    return np.stack([r["y"] for r in res.results], axis=0)


if __name__ == "__main__":
    nc = build_nc()
    print("build OK")
